# revision 10
# baseline (speedup 1.0000x reference)
"""Trainium2 Bass kernel for nn_Attention3D — iteration driver.

Delegates the device body to kernel_v1_body with the current best flag set.
(Inlined into a self-contained file before shipping.)
"""
from contextlib import ExitStack

import numpy as np
import ml_dtypes

import concourse.tile as tile
from concourse import bacc, mybir
from concourse.bass_utils import run_bass_kernel_spmd

import kernel_v1_body as body

# current best flag set
body.D_AV512 = True
body.D_NORM = True
body.D_NORM2 = True

F32 = mybir.dt.float32
F32R = mybir.dt.float32r
BF16 = mybir.dt.bfloat16

B, C, H, W, D = 2, 256, 16, 16, 16
N = H * W * D
HEADS, DH = 8, 64
HID = HEADS * DH
NCORES = 8

_NC_CACHE = None


def _build():
    global _NC_CACHE
    if _NC_CACHE is not None:
        return _NC_CACHE
    nc = bacc.Bacc("TRN2", target_bir_lowering=False, debug=False, num_devices=NCORES)
    XDT = BF16 if body.D_NORM else F32R
    xb = nc.dram_tensor("xb", [C, N], XDT, kind="ExternalInput").ap()
    wa = nc.dram_tensor("wa", [C, 256], XDT, kind="ExternalInput").ap()
    wv = nc.dram_tensor("wv", [C, 256], XDT, kind="ExternalInput").ap()
    wo = nc.dram_tensor("wo", [128, C], F32R, kind="ExternalInput").ap()
    y = nc.dram_tensor("y", [2, C, N], F32, kind="ExternalOutput").ap()
    z = nc.dram_tensor("z", [2, N], F32, kind="ExternalOutput").ap()
    with tile.TileContext(nc) as tc, ExitStack() as ctx:
        with nc.allow_low_precision(reason="fp8 attention within tolerance"):
            body.build_attention_body(nc, tc, ctx, xb, wa, wv, wo, y, z)
    nc.compile()
    _NC_CACHE = nc
    return nc


def _host_prep(x, g, w_qkv, w_out):
    """Per-core input maps."""
    x = np.ascontiguousarray(np.asarray(x, np.float32))
    g = np.asarray(g, np.float32)
    w_qkv = np.asarray(w_qkv, np.float32)
    w_out = np.asarray(w_out, np.float32)

    Wg = w_qkv * (g * np.sqrt(np.float32(C)))[None, :]
    Wq = Wg[0:HID] * np.float32(DH ** -0.5)
    Wk = Wg[HID:2 * HID]
    Wv = Wg[2 * HID:3 * HID]

    in_maps = []
    for core in range(NCORES):
        b = core // 4
        h0 = 2 * (core % 4)
        sl = slice(h0 * DH, (h0 + 2) * DH)
        W_A = np.concatenate([Wq[sl], Wk[sl]], 0)            # [256, 256]
        wo_slice = w_out[:, sl]                              # [256, 128]
        wo_dev = np.ascontiguousarray(
            wo_slice.T.reshape(2, DH, C).transpose(1, 0, 2).reshape(128, C))
        xb_core = x[b].reshape(C, N)
        wa_core = np.ascontiguousarray(W_A.T)
        wv_core = np.ascontiguousarray(np.pad(Wv[sl].T, ((0, 0), (0, 128))))
        if body.D_NORM:
            bf = ml_dtypes.bfloat16
            xb_core = xb_core.astype(bf)
            wa_core = wa_core.astype(bf)
            wv_core = wv_core.astype(bf)
        in_maps.append({
            "xb": np.ascontiguousarray(xb_core),
            "wa": wa_core,                                   # [c, o]
            "wv": wv_core,
            "wo": wo_dev,                                    # [(d,h), c]
        })
    return in_maps


_RUNNER_CACHE = None


def _make_runner(nc):
    """Build the sharded PJRT callable once; reuse across kernel() calls."""
    import jax
    from jax.sharding import Mesh, PartitionSpec
    from jax.experimental.shard_map import shard_map
    from concourse import bass2jax

    bass2jax.install_neuronx_cc_hook()
    in_names, out_names, out_avals, zero_outs = [], [], [], []
    for alloc in nc.m.functions[0].allocations:
        if not isinstance(alloc, mybir.MemoryLocationSet):
            continue
        name = alloc.memorylocations[0].name
        if alloc.kind == "ExternalInput":
            if nc.partition_id_tensor is None or name != nc.partition_id_tensor.name:
                in_names.append(name)
        elif alloc.kind == "ExternalOutput":
            out_names.append(name)
            shape = tuple(alloc.tensor_shape)
            dtype = mybir.dt.np(alloc.dtype)
            out_avals.append(jax.core.ShapedArray(shape, dtype))
            zero_outs.append(np.zeros(shape, dtype))
    n_params = len(in_names)
    all_in_names = list(in_names) + list(out_names)
    if nc.partition_id_tensor is not None:
        all_in_names.append(nc.partition_id_tensor.name)

    def _body(*args):
        operands = list(args)
        if nc.partition_id_tensor is not None:
            operands.append(bass2jax.partition_id_tensor())
        return tuple(bass2jax._bass_exec_p.bind(
            *operands,
            out_avals=tuple(out_avals),
            in_names=tuple(all_in_names),
            out_names=tuple(out_names),
            lowering_input_output_aliases=(),
            sim_require_finite=True,
            sim_require_nnan=True,
            nc=nc,
        ))

    devices = jax.devices()[:NCORES]
    mesh = Mesh(np.asarray(devices), ("core",))
    n_outs = len(out_avals)
    fn = jax.jit(
        shard_map(_body, mesh=mesh,
                  in_specs=(PartitionSpec("core"),) * (n_params + n_outs),
                  out_specs=(PartitionSpec("core"),) * n_outs,
                  check_rep=False),
        keep_unused=True,
    )
    sharding = jax.sharding.NamedSharding(mesh, PartitionSpec("core"))
    dev_zero = [jax.device_put(
        np.zeros((NCORES * zz.shape[0], *zz.shape[1:]), zz.dtype), sharding)
        for zz in zero_outs]

    def run(in_maps):
        concat_in = [np.concatenate([np.asarray(m[name]) for m in in_maps], axis=0)
                     for name in in_names]
        dev_in = [jax.device_put(a, sharding) for a in concat_in]
        outs = fn(*dev_in, *dev_zero)
        res = {}
        for nm, o in zip(list(out_names), outs):
            res[nm] = np.asarray(o)
        yc = res["y"].reshape(NCORES, 2, C, N)
        zc = res["z"].reshape(NCORES, 2, N)
        return yc, zc

    return run


def kernel(x, g, w_qkv, w_out, b_out):
    global _RUNNER_CACHE
    nc = _build()
    in_maps = _host_prep(x, g, w_qkv, w_out)
    try:
        if _RUNNER_CACHE is None:
            _RUNNER_CACHE = _make_runner(nc)
        yc, zc = _RUNNER_CACHE(in_maps)
    except Exception:
        res = run_bass_kernel_spmd(nc, in_maps, core_ids=list(range(NCORES)))
        yc = np.stack([res.results[c]["y"] for c in range(NCORES)])
        zc = np.stack([res.results[c]["z"] for c in range(NCORES)])
    y = np.zeros((B, C, N), np.float32)
    for core in range(NCORES):
        b = core // 4
        y[b] += (yc[core] / zc[core][:, None, :]).sum(axis=0)
    y += np.asarray(b_out, np.float32)[None, :, None]
    return y.reshape(B, C, H, W, D)


# revision 12
# speedup vs baseline: 1.0079x; 1.0079x over previous
"""Trainium2 Bass kernel for nn_Attention3D: RMSNorm3D + 8-head attention + out-proj.

Sharding: 16 (b, h) slices over 8 cores -> each core gets one batch b and two
heads (h0, h0+1). Per-core weights are sliced/folded on the host.

Device pipeline per core:
  - x/W_qkv land in bf16 (halves input DMA; x^2 runs at the 2x DVE rate)
  - norms: per-chunk x^2 + tiny PE ones-reductions into one [128, 32] PSUM
    column tile; a single ACT Sqrt + DVE reciprocal produce 1/||x|| per key,
    then DVE 32x32 stream-transposes + one SBUF DMA flatten it onto
    partition 0 and the idle GPSIMD partition-broadcasts it into the
    [128, N] per-query scale slab (no PE broadcast matmuls, no per-chunk
    sqrt table churn on ACT)
  - qkv projections in bf16; q normalized via inv_bcast, k raw (its
    per-key 1/||x|| rides the exp scale operand), v normalized per-key +
    fp8e4 with a ones column (denominator row)
  - scores s' = k^T q in fp32r (fp8 q/k measured at 3.3e-2 rel err - over
    the 2e-2 budget, so fp32r stays)
  - softmax exp split across two engines (ACT evens + extras, DVE odds):
      ACT tiles: p = Exp(s' * invn + bias) -> fp8e4 (true exp)
      DVE tiles: p = bits_u8(round(max(s' * A/||x||, 0) + B)) Schraudolph
  - attn@V in fp8 DoubleRow (512-wide chunks, 0.5 cyc/row); stationary
    [128, 2, 128] = [v_h | ones | pad]; row 64 of o accumulates Z
  - out-proj per head in fp32r on the UNNORMALIZED o; host divides by Z
Host: y[b] = sum_h (y_partial[core, h] / Z[core, h]) + b_out.
"""
from contextlib import ExitStack

import numpy as np
import ml_dtypes

import concourse.bass as bass
import concourse.tile as tile
from concourse import bacc, mybir
from concourse.bass_utils import run_bass_kernel_spmd

F32 = mybir.dt.float32
F32R = mybir.dt.float32r
BF16 = mybir.dt.bfloat16
F8 = mybir.dt.float8e4
U8 = mybir.dt.uint8
AF = mybir.ActivationFunctionType
Alu = mybir.AluOpType
DR = mybir.MatmulPerfMode.DoubleRow

B, C, H, W, D = 2, 256, 16, 16, 16
N = H * W * D
HEADS, DH = 8, 64
HID = HEADS * DH
NCORES = 8
ICH = 1024
NIC = N // ICH
NJ = N // 128
NG = NJ // 2

A_SCH = float(8 * np.log2(np.e))
SH = 2.0
B_DVE = 56.0 - A_SCH * SH
ACT_DELTA = 0.06
BIAS_ACT = -SH + ACT_DELTA
E4M3_ONE = 0x38
ONE_F32_BITS = 0x3F800000
U32 = mybir.dt.uint32

ACT_SET_EVEN = frozenset(set(range(0, NJ, 2)) | {1, 17})
ACT_SET_ODD = frozenset(set(range(0, NJ, 2)) | {1})
ACT_SET_ODD2 = frozenset(set(range(0, NJ, 2)) | {1, 9})

# retuned sets (D_EXPSET): DVE gets 14/15 per stage
DVE_SET_EVEN = frozenset({0, 2, 4, 6, 8, 11, 13, 15, 17, 19, 21, 24, 26, 28, 30})
DVE_SET_ODD = frozenset({1, 3, 5, 8, 10, 12, 15, 17, 19, 22, 24, 26, 29, 31})

D_AV512 = True
D_NORM = True
D_EXPSET = False
D_AUXPSUM = False
D_TAIL = True
D_EXPSET2 = False
D_NORM2 = True
D_QEARLY = True
D_KEARLY = True
D_TAIL2 = False
D_ACT1ST = True
D_QKPAIR = False
PQ_BUFS = 15
V8_DEFER = 4


def _copy(eng, nc, out, in_):
    if eng is nc.scalar:
        nc.scalar.copy(out, in_)
    else:
        eng.tensor_copy(out, in_)


def build_attention_body(nc, tc, ctx, xb, wa, wv, wo, y, z):
    const = ctx.enter_context(tc.tile_pool(name="const", bufs=1))
    work = ctx.enter_context(tc.tile_pool(name="work", bufs=2))
    pwork = ctx.enter_context(tc.tile_pool(name="pwork", bufs=3))
    psum = ctx.enter_context(tc.tile_pool(name="psum", bufs=2, space="PSUM"))

    XDT = BF16 if D_NORM else F32R
    x_sb = const.tile([128, 2, N], XDT, tag="x")
    wa_sb = const.tile([128, 2, 256], XDT, tag="wa")
    wv_sb = const.tile([128, 2, 256], XDT, tag="wv")
    wo_sb = const.tile([64, 2, 256], F32R, tag="wo")
    ones_col = const.tile([128, 1], XDT, tag="onesc")
    ones_row = const.tile([1, 128], F32R, tag="onesr")
    invn_row = const.tile([1, N], F32 if D_NORM else F32R, tag="invr")
    inv_bcast = const.tile([128, N], F32, tag="invb")
    invn_col = const.tile([128, NJ], F32, tag="invc")
    ainv_col = const.tile([128, NJ], F32, tag="ainvc")
    nrm_col = const.tile([128, NJ], F32, tag="nrmc")
    bias_act = const.tile([128, 1], F32, tag="bact")
    qk_slab = const.tile([128, 2, N], F32R, tag="qk")
    v8 = const.tile([128, NG, 2, 2, 128], F8, tag="v8")
    zpad = const.tile([128, 2, 128], F8, tag="zpad")

    STAG = "s"
    AUX = "aux" if D_AUXPSUM else "s"
    SBUFS = 2 if D_AUXPSUM else 3
    ABUFS = 2 if D_AUXPSUM else 3

    nc.vector.memset(ones_col[:], 1.0)
    nc.vector.memset(ones_row[:].bitcast(U32), ONE_F32_BITS)
    nc.vector.memset(bias_act[:], BIAS_ACT)
    nc.gpsimd.memset(zpad[:].bitcast(U8), 0)
    nc.gpsimd.memset(v8[:, :, :, :, 64:65].bitcast(U8), E4M3_ONE)

    _dma_eng = [nc.sync, nc.gpsimd, nc.scalar, nc.sync]
    for ch in range(8):
        for ct in range(2):
            _dma_eng[(2 * ch + ct) % 4].dma_start(
                out=x_sb[:, ct, ch * 512:(ch + 1) * 512],
                in_=xb[ct * 128:(ct + 1) * 128, ch * 512:(ch + 1) * 512],
            )
    for ct in range(2):
        nc.sync.dma_start(out=wa_sb[:, ct, :], in_=wa[ct * 128:(ct + 1) * 128, :])
        nc.gpsimd.dma_start(out=wv_sb[:, ct, :], in_=wv[ct * 128:(ct + 1) * 128, :])
    nc.sync.dma_start(out=wo_sb[:, :, :], in_=wo.rearrange("(d h) c -> d h c", h=2))

    def norm_bundle(ch, ptag, eng, width=512):
        """v1 norm path (f32r x, ACT sqrt + DVE recip + PE bcast)."""
        sl = slice(ch * 512, ch * 512 + width)
        nw = width // 512
        nr_ps = psum.tile([1, width], F32, tag=ptag, bufs=ABUFS, name=f"nr_ps_{ch}")
        nc_ps = psum.tile([128, 4 * nw], F32, tag=ptag, bufs=ABUFS,
                          name=f"nc_ps_{ch}")
        for w in range(nw):
            wsl = bass.ts(ch + w, 512)
            x2c = [work.tile([128, 512], F32R, tag="x2", bufs=6,
                             name=f"x2_{ch + w}_{i}") for i in range(2)]
            nc.gpsimd.tensor_mul(x2c[0][:], x_sb[:, 0, wsl], x_sb[:, 0, wsl])
            if eng == 0:
                nc.vector.tensor_mul(x2c[1][:], x_sb[:, 1, wsl], x_sb[:, 1, wsl])
            else:
                nc.scalar.activation(out=x2c[1][:], in_=x_sb[:, 1, wsl],
                                     func=AF.Square)
            for ct in range(2):
                nc.tensor.matmul(nr_ps[0:1, w * 512:(w + 1) * 512], ones_col[:],
                                 x2c[ct][:], start=(ct == 0), stop=(ct == 1))
            for tt in range(4):
                for ct in range(2):
                    nc.tensor.matmul(nc_ps[:, 4 * w + tt:4 * w + tt + 1],
                                     x2c[ct][:, tt * 128:(tt + 1) * 128]
                                     .bitcast(F32),
                                     ones_col[:].bitcast(F32),
                                     start=(ct == 0), stop=(ct == 1))
        nrm_c = work.tile([1, width], F32, tag="nr", bufs=3,
                          name=f"nrm_c_{ch}")
        nc.scalar.activation(out=nrm_c[:], in_=nr_ps[:], func=AF.Sqrt)
        nc.vector.reciprocal(out=invn_row[0:1, sl], in_=nrm_c[:])
        csl = slice(ch * 4, ch * 4 + 4 * nw)
        nc.scalar.activation(out=nrm_col[:, csl], in_=nc_ps[:], func=AF.Sqrt)
        nc.vector.reciprocal(out=invn_col[:, csl], in_=nrm_col[:, csl])
        nc.vector.tensor_scalar_mul(out=ainv_col[:, csl], in0=invn_col[:, csl],
                                    scalar1=A_SCH)
        for w in range(nw):
            wsl = bass.ts(ch + w, 512)
            ib_ps = psum.tile([128, 512], F32, tag=ptag, bufs=ABUFS,
                              name=f"ib_ps_{ch + w}")
            nc.tensor.matmul(ib_ps[:], ones_row[:], invn_row[0:1, wsl])
            nc.vector.tensor_copy(inv_bcast[:, wsl], ib_ps[:])

    def norm_chunk2(ch, nc_col):
        """D_NORM2: x^2 only + column-form ones-reductions into the shared
        [128, NJ] psum tile; the single Sqrt/recip/transpose/broadcast
        finalizer runs once after the last chunk."""
        def emit():
            sl = bass.ts(ch, 512)
            x2p = work.tile([128, 2, 512], BF16, tag="x2", bufs=3,
                            name=f"x2_{ch}")
            nc.vector.tensor_mul(x2p[:], x_sb[:, :, sl], x_sb[:, :, sl])
            for tt in range(4):
                for ct in range(2):
                    nc.tensor.matmul(nc_col[:, ch * 4 + tt:ch * 4 + tt + 1],
                                     x2p[:, ct, tt * 128:(tt + 1) * 128],
                                     ones_col[:], start=(ct == 0),
                                     stop=(ct == 1))
        return emit

    def finalize_norms(nc_col, invt):
        def emit():
            scol = work.tile([128, NJ], F32, tag="scol", bufs=1, name="scol")
            nc.scalar.activation(out=scol[:], in_=nc_col[:], func=AF.Sqrt)
            nc.vector.reciprocal(out=invn_col[:, :], in_=scol[:])
            nc.vector.tensor_scalar_mul(out=ainv_col[:, :], in0=invn_col[:, :],
                                        scalar1=A_SCH)
            for rb in range(4):
                nc.vector.transpose(invt[0:32, rb * 32:(rb + 1) * 32],
                                    invn_col[rb * 32:(rb + 1) * 32, 0:32])
            # flatten the [32, 128] transpose onto partition 0: the Q7
            # partition-broadcast kernel only reads partition 0
            nc.sync.dma_start(out=invn_row[0:1, :], in_=invt[:, :])
        return emit

    def bcast_block(c, invt):
        def emit():
            nc.gpsimd.partition_broadcast(
                inv_bcast[:, c * 512:(c + 1) * 512],
                invn_row[0:1, c * 512:(c + 1) * 512], 128)
        return emit

    def norm_chunk(ch):
        """D_NORM path: bf16 x^2, DVE pow(-1/2), gpsimd partition broadcast."""
        def emit():
            sl = bass.ts(ch, 512)
            x2p = work.tile([128, 2, 512], BF16, tag="x2", bufs=3,
                            name=f"x2_{ch}")
            nc.vector.tensor_mul(x2p[:], x_sb[:, :, sl], x_sb[:, :, sl])
            nr_ps = psum.tile([1, 512], F32, tag=AUX, bufs=ABUFS,
                              name=f"nr_ps_{ch}")
            for ct in range(2):
                nc.tensor.matmul(nr_ps[0:1, :], ones_col[:], x2p[:, ct, :],
                                 start=(ct == 0), stop=(ct == 1))
            nrow = work.tile([1, 512], F32, tag="nrow", bufs=3,
                             name=f"nrow_{ch}")
            nc.scalar.activation(out=nrow[:], in_=nr_ps[:], func=AF.Sqrt)
            nc.vector.reciprocal(out=invn_row[0:1, sl], in_=nrow[:])
            nc.gpsimd.partition_broadcast(inv_bcast[:, sl],
                                          invn_row[0:1, sl], 128)
            nc_ps = psum.tile([128, 4], F32, tag=AUX, bufs=ABUFS,
                              name=f"nc_ps_{ch}")
            for tt in range(4):
                for ct in range(2):
                    nc.tensor.matmul(nc_ps[:, tt:tt + 1],
                                     x2p[:, ct, tt * 128:(tt + 1) * 128],
                                     ones_col[:], start=(ct == 0),
                                     stop=(ct == 1))
            csl = slice(ch * 4, ch * 4 + 4)
            ncol = work.tile([128, 4], F32, tag="ncol", bufs=3,
                             name=f"ncol_{ch}")
            nc.scalar.activation(out=ncol[:], in_=nc_ps[:], func=AF.Sqrt)
            nc.vector.reciprocal(out=invn_col[:, csl], in_=ncol[:])
            nc.vector.tensor_scalar_mul(out=ainv_col[:, csl],
                                        in0=invn_col[:, csl], scalar1=A_SCH)
        return emit

    def qk_pair(ch0, which, ptag, eng):
        """q or k projection for chunks ch0, ch0+1 in one [128,1024] psum pair."""
        def emit():
            sl = slice(ch0 * 512, ch0 * 512 + 1024)
            osl = slice(which * 128, which * 128 + 128)
            qk_ps = psum.tile([128, 1024], F32, tag=ptag, bufs=ABUFS,
                              name=f"qkp_{ch0}_{which}")
            for w in range(2):
                for ct in range(2):
                    nc.tensor.matmul(qk_ps[:, w * 512:(w + 1) * 512],
                                     wa_sb[:, ct, osl],
                                     x_sb[:, ct, bass.ts(ch0 + w, 512)],
                                     start=(ct == 0), stop=(ct == 1))
            if which == 0:
                nc.vector.tensor_mul(qk_slab[:, 0, sl], qk_ps[:],
                                     inv_bcast[:, sl])
            else:
                _copy(eng, nc, qk_slab[:, 1, sl], qk_ps[:])
        return emit

    def qk_bundle(ch, which, ptag, eng):
        def emit():
            sl = bass.ts(ch, 512)
            osl = slice(which * 128, which * 128 + 128)
            qk_ps = psum.tile([128, 512], F32, tag=ptag, bufs=ABUFS,
                              name=f"qk_ps_{ch}_{which}")
            for ct in range(2):
                nc.tensor.matmul(qk_ps[:], wa_sb[:, ct, osl],
                                 x_sb[:, ct, sl], start=(ct == 0), stop=(ct == 1))
            if which == 0:
                nc.vector.tensor_mul(qk_slab[:, 0, sl], qk_ps[:],
                                     inv_bcast[:, sl])
            else:
                _copy(eng, nc, qk_slab[:, 1, sl], qk_ps[:])
        return emit

    def v_bundle(t, ptag, eng):
        def emit():
            v_ps = psum.tile([128, 256], F32, tag=ptag, bufs=ABUFS,
                             name=f"v_ps_{t}")
            for ct in range(2):
                nc.tensor.matmul(v_ps[:], x_sb[:, ct, t * 128:(t + 1) * 128],
                                 wv_sb[:, ct, :], start=(ct == 0), stop=(ct == 1))
            if eng is nc.scalar:
                nc.scalar.activation(
                    out=v8[:, t // 2, t % 2, :, 0:64].bitcast(F8),
                    in_=v_ps[:, 0:128].rearrange("p (h d) -> p h d", h=2),
                    func=AF.Copy, scale=invn_col[:, t:t + 1])
            else:
                eng.tensor_scalar_mul(
                    out=v8[:, t // 2, t % 2, :, 0:64].bitcast(F8),
                    in0=v_ps[:, 0:128].rearrange("p (h d) -> p h d", h=2),
                    scalar1=invn_col[:, t:t + 1])
        return emit

    def outproj_piece(ic, o_slab, hh, mt):
        def emit():
            y_ps = psum.tile([128, ICH], F32, tag=AUX if D_AUXPSUM else "s",
                             bufs=ABUFS, name=f"y_ps_{ic}_{hh}_{mt}")
            for cc in range(2):
                nc.tensor.matmul(
                    y_ps[:, cc * 512:(cc + 1) * 512],
                    wo_sb[:, hh, mt * 128:(mt + 1) * 128],
                    o_slab[0:64, hh, cc * 512:(cc + 1) * 512])
            y_ev = pwork.tile([128, ICH], F32, tag="yev", bufs=4,
                              name=f"y_ev_{ic}_{hh}_{mt}")
            eng = nc.vector if (mt + hh) % 2 else nc.scalar
            _copy(eng, nc, y_ev[:], y_ps[:])
            nc.sync.dma_start(
                out=y[hh, mt * 128:(mt + 1) * 128, ic * ICH:(ic + 1) * ICH],
                in_=y_ev[:])
        return emit

    _cv = [nc.vector, nc.scalar]
    if D_NORM2:
        nc_col = psum.tile([128, NJ], F32, tag="o", bufs=1, name="nc_col")
        invt = work.tile([32, 128], F32, tag="invt", bufs=1, name="invt")
        for ch in range(8):
            norm_chunk2(ch, nc_col)()
        finalize_norms(nc_col, invt)()
        for c in range(8):
            bcast_block(c, invt)()
        _pcv = (lambda i: nc.vector) if D_ACT1ST else (lambda i: _cv[i])
        for ch in (0, 1):
            qk_bundle(ch, 1, AUX, _pcv(ch % 2))()
            qk_bundle(ch, 0, AUX, _pcv((ch + 1) % 2))()
            for t in range(4 * ch, 4 * ch + 4):
                v_bundle(t, AUX, _pcv(t % 2))()
    elif D_NORM:
        norm_chunk(0)()
        norm_chunk(1)()
        for ch in (0, 1):
            qk_bundle(ch, 1, AUX, _cv[ch % 2])()
            qk_bundle(ch, 0, AUX, _cv[(ch + 1) % 2])()
            for t in range(4 * ch, 4 * ch + 4):
                v_bundle(t, AUX, _cv[t % 2])()
    else:
        norm_bundle(0, AUX, 0, width=1024)
        for ch in (0, 1):
            qk_bundle(ch, 1, AUX, _cv[ch % 2])()
            qk_bundle(ch, 0, AUX, _cv[(ch + 1) % 2])()
            for t in range(4 * ch, 4 * ch + 4):
                v_bundle(t, AUX, _cv[t % 2])()
        norm_bundle(2, AUX, 0, width=1024)
        norm_bundle(4, AUX, 0, width=1024)
        norm_bundle(6, AUX, 0, width=1024)

    deferred = {}

    def _defer(key, *fns):
        deferred.setdefault(key, []).extend(fns)

    if D_NORM and not D_NORM2:
        for ch in range(2, 8):
            _defer((0, max(0, 2 * ch - 9)), norm_chunk(ch))
    for ch in range(2, 8):
        _defer((0, max(0, (4 * ch - 10) if D_KEARLY else (4 * ch - 6))),
               qk_bundle(ch, 1, AUX, _cv[ch % 2]))
    for t in range(8, 32):
        _defer((0, min(t + V8_DEFER, 29)), v_bundle(t, AUX, _cv[t % 2]))
    for ch in range(2, 8):
        if D_QEARLY:
            key = (0, 20 + 3 * (ch % 2)) if ch < 4 else \
                  (2 * (ch // 2) - 2, 6 + 6 * (ch % 2))
        else:
            key = (2 * (ch // 2) - 1, 6 + 6 * (ch % 2))
        _defer(key, qk_bundle(ch, 0, AUX, _cv[(ch + 1) % 2]))

    stages = [(ic, h) for ic in range(NIC) for h in range(2)]
    o_slab = None
    for si, (ic, h) in enumerate(stages):
        if h == 0:
            o_slab = work.tile([65, 2, ICH], F32R, tag="osl", bufs=3,
                               name=f"osl_{ic}")
        hsl = slice(h * 64, (h + 1) * 64)
        o_ps = psum.tile([128, ICH], F32, tag="o", bufs=1,
                         name=f"o_ps_{ic}_{h}")
        pq = {}
        last = si == len(stages) - 1
        av_j0 = 5 if (D_TAIL and last) else 9
        for j in range(NJ):
            g, pl = j // 2, j % 2
            if pl == 0:
                pq[g] = pwork.tile([128, 2, ICH], U8, tag="p", bufs=PQ_BUFS,
                                   name=f"p_{ic}_{h}_{g}")
            s_ps = psum.tile([128, ICH], F32, tag=STAG, bufs=SBUFS,
                             name=f"s_{ic}_{h}_{j}")
            for hf in range(2):
                nc.tensor.matmul(
                    s_ps[:, hf * 512:(hf + 1) * 512],
                    qk_slab[hsl, 1, j * 128:(j + 1) * 128],
                    qk_slab[hsl, 0, ic * ICH + hf * 512:ic * ICH + (hf + 1) * 512])
            for fn in deferred.pop((si, j), []):
                fn()
            if j == av_j0:
                if D_AV512:
                    for qc in range(2):
                        nc.tensor.matmul(
                            o_ps[:, qc * 512:(qc + 1) * 512],
                            zpad[:],
                            pq[0][:, :, qc * 512:(qc + 1) * 512].bitcast(F8),
                            perf_mode=DR, start=True, stop=False)
                else:
                    for qc in range(4):
                        nc.tensor.matmul(
                            o_ps[:, qc * 256:(qc + 1) * 256],
                            zpad[:],
                            pq[0][:, :, qc * 256:(qc + 1) * 256].bitcast(F8),
                            perf_mode=DR, start=True, stop=False)
            if j >= av_j0 and (j - av_j0) % 2 == 0 and (j - av_j0) // 2 < NG - 4:
                ag = (j - av_j0) // 2
                if D_AV512:
                    for qc in range(2):
                        nc.tensor.matmul(
                            o_ps[:, qc * 512:(qc + 1) * 512],
                            v8[:, ag, :, h, :],
                            pq[ag][:, :, qc * 512:(qc + 1) * 512].bitcast(F8),
                            perf_mode=DR, start=False, stop=False)
                else:
                    for qc in range(4):
                        nc.tensor.matmul(
                            o_ps[:, qc * 256:(qc + 1) * 256],
                            v8[:, ag, :, h, :],
                            pq[ag][:, :, qc * 256:(qc + 1) * 256].bitcast(F8),
                            perf_mode=DR, start=False, stop=False)
            if D_EXPSET:
                dve = (DVE_SET_EVEN if si % 2 == 0 else DVE_SET_ODD)
                use_act = j not in dve
            else:
                act_set = ACT_SET_EVEN if si % 2 == 0 else (
                    ACT_SET_ODD2 if D_EXPSET2 else ACT_SET_ODD)
                use_act = j in act_set
            if use_act:
                nc.scalar.activation(out=pq[g][:, pl, :].bitcast(F8),
                                     in_=s_ps[:], func=AF.Exp, bias=bias_act[:],
                                     scale=invn_col[:, j:j + 1])
            else:
                nc.vector.tensor_scalar(out=pq[g][:, pl, :], in0=s_ps[:],
                                        scalar1=ainv_col[:, j:j + 1],
                                        scalar2=B_DVE,
                                        op0=Alu.mult, op1=Alu.add)

        def tail_avs(h, o_ps, pq):
            def emit():
                for ag in range(NG - 4, NG):
                    if D_AV512:
                        for qc in range(2):
                            nc.tensor.matmul(
                                o_ps[:, qc * 512:(qc + 1) * 512],
                                v8[:, ag, :, h, :],
                                pq[ag][:, :, qc * 512:(qc + 1) * 512].bitcast(F8),
                                perf_mode=DR, start=False,
                                stop=(ag == NG - 1 and qc == 1))
                    else:
                        for qc in range(4):
                            nc.tensor.matmul(
                                o_ps[:, qc * 256:(qc + 1) * 256],
                                v8[:, ag, :, h, :],
                                pq[ag][:, :, qc * 256:(qc + 1) * 256].bitcast(F8),
                                perf_mode=DR, start=False, stop=(ag == NG - 1))
            return emit

        def o_copy(h, ic, o_ps, o_slab, split=False):
            def emit():
                if split:
                    nc.vector.tensor_copy(o_slab[:, h, 0:512], o_ps[0:65, 0:512])
                    nc.scalar.copy(o_slab[:, h, 512:1024], o_ps[0:65, 512:1024])
                else:
                    eng = nc.vector if h else nc.scalar
                    _copy(eng, nc, o_slab[:, h, :], o_ps[0:65, :])
                nc.sync.dma_start(out=z[h, ic * ICH:(ic + 1) * ICH],
                                  in_=o_slab[64:65, h, :].bitcast(F32))
            return emit

        deferred.setdefault((si + 1, 1), []).append(tail_avs(h, o_ps, pq))
        deferred.setdefault((si + 1, 3), []).append(
            o_copy(h, ic, o_ps, o_slab, split=(D_TAIL and last)))
        if D_TAIL2:
            for pi, mt in enumerate([0, 1]):
                deferred.setdefault((si + 1, 13 + 8 * pi), []).append(
                    outproj_piece(ic, o_slab, h, mt))
        elif h == 1:
            for pi, (hh, mt) in enumerate([(0, 0), (0, 1), (1, 0), (1, 1)]):
                deferred.setdefault((si + 1, 13 + 4 * pi), []).append(
                    outproj_piece(ic, o_slab, hh, mt))
    for key in sorted(deferred):
        for fn in deferred[key]:
            fn()


_NC_CACHE = None


def _build():
    global _NC_CACHE
    if _NC_CACHE is not None:
        return _NC_CACHE
    nc = bacc.Bacc("TRN2", target_bir_lowering=False, debug=False, num_devices=NCORES)
    XDT = BF16
    xb = nc.dram_tensor("xb", [C, N], XDT, kind="ExternalInput").ap()
    wa = nc.dram_tensor("wa", [C, 256], XDT, kind="ExternalInput").ap()
    wv = nc.dram_tensor("wv", [C, 256], XDT, kind="ExternalInput").ap()
    wo = nc.dram_tensor("wo", [128, C], F32R, kind="ExternalInput").ap()
    y = nc.dram_tensor("y", [2, C, N], F32, kind="ExternalOutput").ap()
    z = nc.dram_tensor("z", [2, N], F32, kind="ExternalOutput").ap()
    with tile.TileContext(nc) as tc, ExitStack() as ctx:
        with nc.allow_low_precision(reason="fp8 attention within tolerance"):
            build_attention_body(nc, tc, ctx, xb, wa, wv, wo, y, z)
    nc.compile()
    _NC_CACHE = nc
    return nc


def _host_prep(x, g, w_qkv, w_out):
    """Per-core input maps."""
    x = np.ascontiguousarray(np.asarray(x, np.float32))
    g = np.asarray(g, np.float32)
    w_qkv = np.asarray(w_qkv, np.float32)
    w_out = np.asarray(w_out, np.float32)

    Wg = w_qkv * (g * np.sqrt(np.float32(C)))[None, :]
    Wq = Wg[0:HID] * np.float32(DH ** -0.5)
    Wk = Wg[HID:2 * HID]
    Wv = Wg[2 * HID:3 * HID]

    in_maps = []
    for core in range(NCORES):
        b = core // 4
        h0 = 2 * (core % 4)
        sl = slice(h0 * DH, (h0 + 2) * DH)
        W_A = np.concatenate([Wq[sl], Wk[sl]], 0)            # [256, 256]
        wo_slice = w_out[:, sl]                              # [256, 128]
        wo_dev = np.ascontiguousarray(
            wo_slice.T.reshape(2, DH, C).transpose(1, 0, 2).reshape(128, C))
        xb_core = x[b].reshape(C, N)
        wa_core = np.ascontiguousarray(W_A.T)
        wv_core = np.ascontiguousarray(np.pad(Wv[sl].T, ((0, 0), (0, 128))))
        if True:
            bf = ml_dtypes.bfloat16
            xb_core = xb_core.astype(bf)
            wa_core = wa_core.astype(bf)
            wv_core = wv_core.astype(bf)
        in_maps.append({
            "xb": np.ascontiguousarray(xb_core),
            "wa": wa_core,                                   # [c, o]
            "wv": wv_core,
            "wo": wo_dev,                                    # [(d,h), c]
        })
    return in_maps


_RUNNER_CACHE = None


def _make_runner(nc):
    """Build the sharded PJRT callable once; reuse across kernel() calls."""
    import jax
    from jax.sharding import Mesh, PartitionSpec
    from jax.experimental.shard_map import shard_map
    from concourse import bass2jax

    bass2jax.install_neuronx_cc_hook()
    in_names, out_names, out_avals, zero_outs = [], [], [], []
    for alloc in nc.m.functions[0].allocations:
        if not isinstance(alloc, mybir.MemoryLocationSet):
            continue
        name = alloc.memorylocations[0].name
        if alloc.kind == "ExternalInput":
            if nc.partition_id_tensor is None or name != nc.partition_id_tensor.name:
                in_names.append(name)
        elif alloc.kind == "ExternalOutput":
            out_names.append(name)
            shape = tuple(alloc.tensor_shape)
            dtype = mybir.dt.np(alloc.dtype)
            out_avals.append(jax.core.ShapedArray(shape, dtype))
            zero_outs.append(np.zeros(shape, dtype))
    n_params = len(in_names)
    all_in_names = list(in_names) + list(out_names)
    if nc.partition_id_tensor is not None:
        all_in_names.append(nc.partition_id_tensor.name)

    def _body(*args):
        operands = list(args)
        if nc.partition_id_tensor is not None:
            operands.append(bass2jax.partition_id_tensor())
        return tuple(bass2jax._bass_exec_p.bind(
            *operands,
            out_avals=tuple(out_avals),
            in_names=tuple(all_in_names),
            out_names=tuple(out_names),
            lowering_input_output_aliases=(),
            sim_require_finite=True,
            sim_require_nnan=True,
            nc=nc,
        ))

    devices = jax.devices()[:NCORES]
    mesh = Mesh(np.asarray(devices), ("core",))
    n_outs = len(out_avals)
    fn = jax.jit(
        shard_map(_body, mesh=mesh,
                  in_specs=(PartitionSpec("core"),) * (n_params + n_outs),
                  out_specs=(PartitionSpec("core"),) * n_outs,
                  check_rep=False),
        keep_unused=True,
    )
    sharding = jax.sharding.NamedSharding(mesh, PartitionSpec("core"))
    dev_zero = [jax.device_put(
        np.zeros((NCORES * zz.shape[0], *zz.shape[1:]), zz.dtype), sharding)
        for zz in zero_outs]

    def run(in_maps):
        concat_in = [np.concatenate([np.asarray(m[name]) for m in in_maps], axis=0)
                     for name in in_names]
        dev_in = [jax.device_put(a, sharding) for a in concat_in]
        outs = fn(*dev_in, *dev_zero)
        res = {}
        for nm, o in zip(list(out_names), outs):
            res[nm] = np.asarray(o)
        yc = res["y"].reshape(NCORES, 2, C, N)
        zc = res["z"].reshape(NCORES, 2, N)
        return yc, zc

    return run


def kernel(x, g, w_qkv, w_out, b_out):
    global _RUNNER_CACHE
    nc = _build()
    in_maps = _host_prep(x, g, w_qkv, w_out)
    try:
        if _RUNNER_CACHE is None:
            _RUNNER_CACHE = _make_runner(nc)
        yc, zc = _RUNNER_CACHE(in_maps)
    except Exception:
        res = run_bass_kernel_spmd(nc, in_maps, core_ids=list(range(NCORES)))
        yc = np.stack([res.results[c]["y"] for c in range(NCORES)])
        zc = np.stack([res.results[c]["z"] for c in range(NCORES)])
    y = np.zeros((B, C, N), np.float32)
    for core in range(NCORES):
        b = core // 4
        y[b] += (yc[core] / zc[core][:, None, :]).sum(axis=0)
    y += np.asarray(b_out, np.float32)[None, :, None]
    return y.reshape(B, C, H, W, D)
